# revision 32
# baseline (speedup 1.0000x reference)
"""Trainium2 Bass kernel for nn_BaseTransformer (B=16, C=128, L=1024, H=8, dk=dv=32).

Sharding: pure data-parallel over batch — 8 cores x 2 batches each, no collectives.

The ACT engine (the only engine that can run Exp) is the hard bottleneck:
16.8M softmax exps per core = 128 activation instructions x ~1.04us = ~133us
busy. Everything else is scheduled around keeping it 100% occupied; the
steady-state (marginal KREPEAT pass) sits within ~0.3% of that floor:
  - one global software-pipelined stream over (batch, j-half, head-group)
    blocks of 8 key-chunks each: QK logits matmuls run exactly one chunk
    ahead of the exp stream, PV consumes one chunk behind, and each batch's
    q/k/v projections are injected as half-chunk units into fixed slack
    slots (i in {1,3,4,6}) of the previous batch's blocks — including
    across KREPEAT boundaries, so the marginal pass has no prologue.
  - PSUM discipline: the QK `pl` ring (2 x 2 banks) is touched by nothing
    else, PV accumulators rotate a 3 x 1-bank ring, injected projections
    and W_o use a dedicated 1-bank slot (8 banks total). Any foreign
    allocation in the pl ring displaces the double-buffer and costs an exp
    bubble; GPSIMD cannot access PSUM at all, so every PSUM reader is DVE.
  - j-outer block order: W_o + bias + output-DMA of a finished (b, j) half
    run inside a later block's slack slot; the last block normalizes and
    projects in a shortened tail.
  - PE warmup matmuls during the initial DMAs climb the pstate ladder
    (cold PE runs at ~1/2 clock for 3us); a dummy exp preloads the ACT
    exp table; DMA order puts wqk and the first x half-tile first.

Numerics (bf16 PE datapath, fp32 PSUM + softmax normalization): q/k/v
projections bf16; SCALE and q-bias folded host-side; k-bias dropped (softmax-
invariant); v-bias folded into the output bias via W_o @ b_v; logits computed
transposed (S^T[t,s]); softmax denominator produced by the PV matmul against
an all-ones block of the stationary (denmerge).
"""

import os
import numpy as np

B, C, L = 16, 128, 1024
DK, DV, H = 32, 32, 8
SCALE = DK ** (-0.5)
NCORES = 8
BLOC = B // NCORES  # batches per core

_CACHE = {}


def _split_excess_waits(nc, mybir, cap=1):
    """This container's walrus rejects instructions carrying more than one
    sync-wait command ("Too many sync wait commands" in setupSyncWait), while
    Tile freely attaches several. Move all but `cap` waits of every
    instruction onto injected same-engine NoOps placed immediately before it
    (same block order == same engine queue order, so semantics are identical:
    all waits still complete before the instruction issues)."""
    ctr = 0
    for f in nc.m.functions:
        for blk in f.blocks:
            out = []
            changed = False
            for ins in blk.instructions:
                si = ins.sync_info
                waits = list(si.on_wait) if si and si.on_wait else []
                eng = getattr(ins, "engine", None)
                if len(waits) > cap and eng is not None:
                    for w in waits[:-cap]:
                        nop = mybir.InstNoOp(name=f"I-wsplit-{ctr}")
                        ctr += 1
                        nop.engine = eng
                        nop.sync_info = mybir.SyncInfo(on_wait=[w], on_update=[])
                        out.append(nop)
                    ins.sync_info = mybir.SyncInfo(
                        on_wait=waits[-cap:], on_update=list(si.on_update or [])
                    )
                    changed = True
                out.append(ins)
            if changed:
                blk.instructions = out
    return nc


def _build_nc():
    import concourse.bass as bass
    import concourse.tile as tile
    from concourse import mybir
    from contextlib import ExitStack

    f32 = mybir.dt.float32
    bf16 = mybir.dt.bfloat16
    nc = bass.Bass()

    x_d = nc.dram_tensor("x_sh", [BLOC, C, L], bf16, kind="ExternalInput")
    wqk_d = nc.dram_tensor("wqk", [C, 4, 128], bf16, kind="ExternalInput")
    bqk_d = nc.dram_tensor("bqk", [128, 2], f32, kind="ExternalInput")
    wv_d = nc.dram_tensor("wv", [C, 256], bf16, kind="ExternalInput")
    wo_d = nc.dram_tensor("wo", [128, 3, 128], bf16, kind="ExternalInput")
    bout_d = nc.dram_tensor("bout", [128, 1], f32, kind="ExternalInput")
    out_d = nc.dram_tensor("out_sh", [BLOC, C, L], f32, kind="ExternalOutput")

    Exp = mybir.ActivationFunctionType.Exp
    mult = mybir.AluOpType.mult

    with tile.TileContext(nc) as tc, ExitStack() as ctx:
        consts = ctx.enter_context(tc.tile_pool(name="consts", bufs=1))
        xp = ctx.enter_context(tc.tile_pool(name="xp", bufs=3))
        qkp = ctx.enter_context(tc.tile_pool(name="qkp", bufs=2))
        vtp = ctx.enter_context(tc.tile_pool(name="vtp", bufs=2))
        stp = ctx.enter_context(tc.tile_pool(name="stp", bufs=6))
        zfp = ctx.enter_context(tc.tile_pool(name="zfp", bufs=2))
        rbp = ctx.enter_context(tc.tile_pool(name="rbp", bufs=3))
        outp = ctx.enter_context(tc.tile_pool(name="outp", bufs=2))
        # PSUM: pl ring 2 x 2 banks (QK logits, kept pure double-buffer),
        # comb ring 3 x 1 bank (PV accumulators), proj slot 1 x 1 bank
        # (injected projections / W_o halves) = 8 banks.
        pbig = ctx.enter_context(tc.tile_pool(name="pbig", bufs=2, space="PSUM"))
        pacc = ctx.enter_context(tc.tile_pool(name="pacc", bufs=3, space="PSUM"))
        pprj = ctx.enter_context(tc.tile_pool(name="pprj", bufs=1, space="PSUM"))

        wqk_sb = consts.tile([C, 4, 128], bf16, name="wqk_sb")
        bqk_sb = consts.tile([128, 2], f32, name="bqk_sb")
        wv_sb = consts.tile([C, 256], bf16, name="wv_sb")
        wo_sb = consts.tile([128, 3, 128], bf16, name="wo_sb")
        bout_sb = consts.tile([128, 1], f32, name="bout_sb")
        ones_sb = consts.tile([128, 32], bf16, name="ones_sb")
        warm_sb = consts.tile([128, 512], bf16, name="warm_sb")
        tprld = consts.tile([128, 2], bf16, name="tprld")

        repeat = int(os.environ.get("KREPEAT", "1"))
        ALLB = BLOC * repeat            # global batch stream (b = gb % BLOC)
        blocks = [(gb, j, g) for gb in range(ALLB) for j in range(2)
                  for g in range(2)]
        NB = len(blocks)

        bs = {}   # per-global-batch tiles (x, q/k, vt)
        S = {}    # transient chunk state

        def xdma(gb):
            x_sb = xp.tile([C, L], bf16, name="x_sb")
            if gb == 0:
                # halves: the first QK chunk only needs x[:, :512], so the
                # pipeline starts one half-DMA earlier.
                nc.sync.dma_start(out=x_sb[:, 0:512], in_=x_d[0][:, 0:512])
                nc.sync.dma_start(out=x_sb[:, 512:], in_=x_d[0][:, 512:])
            else:
                nc.sync.dma_start(out=x_sb, in_=x_d[gb % BLOC])
            bs[gb] = dict(x_sb=x_sb)

        def proj_qk_half(gb, cch, jh):
            # one [128, 512] half of a q/k projection chunk through the
            # dedicated 1-bank proj slot: matmul + bias/copy immediately.
            x_sb = bs[gb]["x_sb"]
            nm = ("qA", "qB", "kA", "kB")[cch]
            if jh == 0:
                bs[gb][nm] = qkp.tile([128, L], bf16, name=nm)
            t = bs[gb][nm]
            sj = slice(512 * jh, 512 * jh + 512)
            ps = pprj.tile([128, 512], f32, name="pp")
            nc.tensor.matmul(
                out=ps, lhsT=wqk_sb[:, cch, :], rhs=x_sb[:, sj],
                start=True, stop=True,
            )
            if cch < 2:
                nc.vector.tensor_scalar_add(
                    out=t[:, sj], in0=ps, scalar1=bqk_sb[:, cch : cch + 1]
                )
            else:
                nc.vector.tensor_copy(out=t[:, sj], in_=ps)

        def proj_v_half(gb, gq, hf):
            # one [128, 512] half of a v^T projection group (2 x-chunks).
            x_sb = bs[gb]["x_sb"]
            if "vt" not in bs[gb]:
                bs[gb]["vt"] = vtp.tile([128, 8, 8, 64], bf16, name="vt")
            vt = bs[gb]["vt"]
            ps = pprj.tile([128, 512], f32, name="pp")
            for q in range(2):
                cc = 4 * gq + 2 * hf + q
                nc.tensor.matmul(
                    out=ps[:, 256 * q : 256 * q + 256],
                    lhsT=x_sb[:, 128 * cc : 128 * cc + 128],
                    rhs=wv_sb,
                    start=True, stop=True,
                )
            a0 = 4 * gq + 2 * hf
            nc.vector.tensor_copy(
                out=vt[:, a0 : a0 + 2, :, 0:32],
                in_=ps.rearrange("p (a h d) -> p a h d", h=8, d=32),
            )
            if gq == 1 and hf == 1:
                nc.gpsimd.memset(vt[:, :, :, 32:64], 1.0)

        def proj_v(gb):
            # vt[t, i, h, d'] with d' = [v (32) | ones (32)] for denmerge.
            # Four [128, 512] halves through the comb ring — ONLY safe while
            # no PV accumulation is in flight (prologue).
            x_sb = bs[gb]["x_sb"]
            bs[gb]["vt"] = vt = vtp.tile([128, 8, 8, 64], bf16, name="vt")
            for hf in range(4):
                ps = pacc.tile([128, 512], f32, name="comb")
                for q in range(2):
                    cc = 2 * hf + q
                    nc.tensor.matmul(
                        out=ps[:, 256 * q : 256 * q + 256],
                        lhsT=x_sb[:, 128 * cc : 128 * cc + 128],
                        rhs=wv_sb,
                        start=True, stop=True,
                    )
                nc.vector.tensor_copy(
                    out=vt[:, 2 * hf : 2 * hf + 2, :, 0:32],
                    in_=ps.rearrange("p (a h d) -> p a h d", h=8, d=32),
                )
            nc.gpsimd.memset(vt[:, :, :, 32:64], 1.0)

        def qk(m, i):
            gb, j, g = blocks[m]
            q_t = bs[gb][("qA", "qB")[g]]
            k_t = bs[gb][("kA", "kB")[g]]
            sj = slice(512 * j, 512 * j + 512)
            pls = []
            for pp in range(2):
                pl = pbig.tile([128, 2, 512], f32, name="pl")
                for hh in range(2):
                    rr = 64 * pp + 32 * hh
                    nc.tensor.matmul(
                        out=pl[:, hh, :],
                        lhsT=k_t[rr : rr + 32, 128 * i : 128 * i + 128],
                        rhs=q_t[rr : rr + 32, sj],
                        start=True, stop=True,
                        tile_position=(rr, 0),
                    )
                pls.append(pl)
            S[(m, i)] = pls

        def expi(m, i):
            pls = S.pop((m, i))
            sts = []
            for pp in range(2):
                st = stp.tile([128, 2, 512], bf16, name="st")
                nc.scalar.activation(out=st, in_=pls[pp], func=Exp)
                sts.append(st)
            S[(m, i, "st")] = sts

        def pv(m, i):
            gb, j, g = blocks[m]
            vt = bs[gb]["vt"]
            sts = S.pop((m, i, "st"))
            if i == 0:
                S[(m, "comb")] = [
                    pacc.tile([128, 512], f32, name="comb") for _ in range(2)
                ]
            combs = S[(m, "comb")]
            for pp in range(2):
                for hh in range(2):
                    nc.tensor.matmul(
                        out=combs[pp][64 * hh : 64 * hh + 64, :],
                        lhsT=vt[:, i, 4 * g + 2 * pp + hh, :],
                        rhs=sts[pp][:, hh, :],
                        start=(i == 0), stop=(i == 7),
                        tile_position=(0, 64 * hh),
                        skip_group_check=True,
                    )

        def norm(m):
            gb, j, g = blocks[m]
            if ("zf", gb, g) not in S:
                S[("zf", gb, g)] = zfp.tile(
                    [128, L], bf16, name=("zfA", "zfB")[g]
                )
            zf = S[("zf", gb, g)]
            sj = slice(512 * j, 512 * j + 512)
            combs = S.pop((m, "comb"))
            for pp in range(2):
                rb = rbp.tile([128, 512], f32, name="rb")
                nc.vector.reciprocal(out=rb, in_=combs[pp])
                for hh in range(2):
                    h4 = 2 * pp + hh
                    nc.vector.tensor_tensor(
                        out=zf[32 * h4 : 32 * h4 + 32, sj],
                        in0=combs[pp][64 * hh : 64 * hh + 32, :],
                        in1=rb[64 * hh + 32 : 64 * hh + 64, :],
                        op=mult,
                    )

        def wo(gb, j):
            x_sb = bs[gb]["x_sb"]
            zfA = S.pop(("zf", gb, 0)) if j == 1 else S[("zf", gb, 0)]
            zfB = S.pop(("zf", gb, 1)) if j == 1 else S[("zf", gb, 1)]
            sj = slice(512 * j, 512 * j + 512)
            po = pprj.tile([128, 512], f32, name="pp")
            # zfB last: it is the freshest dependency (same-window norm).
            nc.tensor.matmul(out=po, lhsT=wo_sb[:, 0, :], rhs=zfA[:, sj],
                             start=True, stop=False)
            nc.tensor.matmul(out=po, lhsT=wo_sb[:, 2, :], rhs=x_sb[:, sj],
                             start=False, stop=False)
            nc.tensor.matmul(out=po, lhsT=wo_sb[:, 1, :], rhs=zfB[:, sj],
                             start=False, stop=True)
            o_sb = outp.tile([128, 512], f32, name="o_sb")
            # bias + out-DMA in halves so the first DMA starts while the
            # second half is still being written (GPSIMD cannot read PSUM,
            # so both run on DVE).
            bb = gb % BLOC
            for hf in range(2):
                sh = slice(256 * hf, 256 * hf + 256)
                nc.vector.tensor_scalar_add(
                    out=o_sb[:, sh], in0=po[:, sh], scalar1=bout_sb[:, 0:1]
                )
                nc.sync.dma_start(
                    out=out_d[bb][:, 512 * j + 256 * hf : 512 * j + 256 * hf + 256],
                    in_=o_sb[:, sh],
                )
            if j == 1:
                del bs[gb]

        # injection schedule:
        #  - mids: one PE unit at i==2 and one at i==5 of a block (emitted
        #    right after a qk lookahead), carrying the NEXT batch's
        #    projections through the previous batch's window, and batch 0's
        #    own qB/kB through its first block.
        #  - lates (at i==7, after the qk(m+1, 0) lookahead): W_o of the
        #    last completed (gb, j) output half.
        mids = {}
        lates = {}

        def sched_win(b1, g_):
            # 12 projection half-units of batch g_ spread over three blocks
            # of the previous batch's window, 4 per block at i in {1,3,4,6}.
            units = [
                lambda jh: proj_qk_half(g_, 0, jh),
                lambda jh: proj_qk_half(g_, 2, jh),
                lambda jh: proj_v_half(g_, 0, jh),
                lambda jh: proj_v_half(g_, 1, jh),
                lambda jh: proj_qk_half(g_, 1, jh),
                lambda jh: proj_qk_half(g_, 3, jh),
            ]
            for u, unit in enumerate(units):
                blk = b1 + u // 2
                i0, i1 = (1, 3) if u % 2 == 0 else (4, 6)
                mids[(blk, i0)] = lambda unit=unit: unit(0)
                mids[(blk, i1)] = lambda unit=unit: unit(1)

        mids[(0, 1)] = lambda: proj_qk_half(0, 1, 0)   # qB(0)
        mids[(0, 3)] = lambda: proj_qk_half(0, 1, 1)
        mids[(0, 4)] = lambda: proj_qk_half(0, 3, 0)   # kB(0)
        mids[(0, 6)] = lambda: proj_qk_half(0, 3, 1)
        for _gb in range(1, ALLB):
            mids[(4 * (_gb - 1) + 1, 0)] = lambda g_=_gb: xdma(g_)
            sched_win(4 * (_gb - 1) + 1, _gb)
        for _gb in range(ALLB):
            lates[4 * _gb + 2] = lambda g_=_gb: wo(g_, 0)
            if 4 * _gb + 4 < NB:
                lates[4 * _gb + 4] = lambda g_=_gb: wo(g_, 1)

        # ---- prologue. DMA order: wqk (projection weights, the longest
        # pole) -> x0 half0 -> bqk (ACT table preload) -> x0 half1 -> rest.
        # qA/kA projections run as column halves through the pl ring so each
        # bias/copy only waits its own half; five PE-warmup matmuls climb
        # the pstate ladder during the DMAs.
        x_sb0 = xp.tile([C, L], bf16, name="x_sb")
        bs[0] = dict(x_sb=x_sb0)
        nc.sync.dma_start(out=wqk_sb, in_=wqk_d[:, :, :])
        nc.sync.dma_start(out=x_sb0[:, 0:512], in_=x_d[0][:, 0:512])
        nc.sync.dma_start(out=bqk_sb, in_=bqk_d[:, :])
        nc.scalar.activation(out=tprld, in_=bqk_sb, func=Exp)
        nc.sync.dma_start(out=x_sb0[:, 512:], in_=x_d[0][:, 512:])
        nc.sync.dma_start(out=wv_sb, in_=wv_d[:, :])
        nc.sync.dma_start(out=wo_sb, in_=wo_d[:, :, :])
        nc.sync.dma_start(out=bout_sb, in_=bout_d[:, :])
        nc.vector.memset(warm_sb, 0.0)
        nc.vector.memset(ones_sb, 1.0)
        pwarm = pacc.tile([128, 512], f32, name="comb")
        for m in range(6):
            r0 = 32 * (m % 4)
            nc.tensor.matmul(
                out=pwarm[r0 : r0 + 32, :], lhsT=warm_sb[:, 0:32],
                rhs=warm_sb,
                start=True, stop=True, tile_position=(0, r0),
            )
        qA = qkp.tile([128, L], bf16, name="qA")
        kA = qkp.tile([128, L], bf16, name="kA")
        for jh in range(2):
            sj = slice(512 * jh, 512 * jh + 512)
            ps = pbig.tile([128, 512], f32, name="pl")
            nc.tensor.matmul(out=ps, lhsT=wqk_sb[:, 0, :], rhs=x_sb0[:, sj],
                             start=True, stop=True)
            nc.vector.tensor_scalar_add(
                out=qA[:, sj], in0=ps, scalar1=bqk_sb[:, 0:1]
            )
        for jh in range(2):
            sj = slice(512 * jh, 512 * jh + 512)
            ps = pbig.tile([128, 512], f32, name="pl")
            nc.tensor.matmul(out=ps, lhsT=wqk_sb[:, 2, :], rhs=x_sb0[:, sj],
                             start=True, stop=True)
            nc.vector.tensor_copy(out=kA[:, sj], in_=ps)
        bs[0]["qA"] = qA
        bs[0]["kA"] = kA
        qk(0, 0)
        proj_v(0)

        def tail(m):
            # last block: the serial chain after the final exp is
            # pv -> norm (DVE) -> W_o zfB matmul -> bias -> DMA. The zfA and
            # x matmuls run during the norm; bias/DMA split into halves (both
            # DVE: GPSIMD cannot read PSUM) so the first output DMA starts
            # while the second half is still being written.
            gb, j, g = blocks[m]
            zfA = S.pop(("zf", gb, 0))
            zf = S.pop(("zf", gb, g))
            x_sb = bs[gb]["x_sb"]
            combs = S.pop((m, "comb"))
            sj = slice(512 * j, 512 * j + 512)
            for pp in range(2):
                rb = rbp.tile([128, 512], f32, name="rb")
                nc.vector.reciprocal(out=rb, in_=combs[pp])
                for hh in range(2):
                    h4 = 2 * pp + hh
                    nc.vector.tensor_tensor(
                        out=zf[32 * h4 : 32 * h4 + 32, sj],
                        in0=combs[pp][64 * hh : 64 * hh + 32, :],
                        in1=rb[64 * hh + 32 : 64 * hh + 64, :],
                        op=mult,
                    )
            po = pbig.tile([128, 512], f32, name="pl")
            nc.tensor.matmul(out=po, lhsT=wo_sb[:, 0, :], rhs=zfA[:, sj],
                             start=True, stop=False)
            nc.tensor.matmul(out=po, lhsT=wo_sb[:, 2, :], rhs=x_sb[:, sj],
                             start=False, stop=False)
            nc.tensor.matmul(out=po, lhsT=wo_sb[:, 1, :], rhs=zf[:, sj],
                             start=False, stop=True)
            o_sb = outp.tile([128, 512], f32, name="o_sb")
            bb = gb % BLOC
            nc.vector.tensor_scalar_add(
                out=o_sb[:, 0:256], in0=po[:, 0:256], scalar1=bout_sb[:, 0:1]
            )
            nc.sync.dma_start(
                out=out_d[bb][:, 512 * j : 512 * j + 256], in_=o_sb[:, 0:256]
            )
            nc.vector.tensor_scalar_add(
                out=o_sb[:, 256:], in0=po[:, 256:], scalar1=bout_sb[:, 0:1]
            )
            nc.sync.dma_start(
                out=out_d[bb][:, 512 * j + 256 : 512 * j + 512],
                in_=o_sb[:, 256:],
            )

        for m in range(NB):
            for i in range(8):
                if i < 7:
                    qk(m, i + 1)
                else:
                    if m + 1 < NB:
                        qk(m + 1, 0)
                    fn = lates.get(m)
                    if fn is not None:
                        fn()
                fn = mids.get((m, i))
                if fn is not None:
                    fn()
                expi(m, i)
                pv(m, i)
            if m == NB - 1:
                tail(m)
            else:
                norm(m)

    from concourse import mybir as _mybir
    _split_excess_waits(nc, _mybir)
    nc.finalize()
    return nc


def get_nc():
    if "nc" not in _CACHE:
        _CACHE["nc"] = _build_nc()
    return _CACHE["nc"]


def prep_weights(w_qkv, b_qkv, w_o, b_o, w_res, b_res):
    w_qkv = np.asarray(w_qkv, np.float32)
    b_qkv = np.asarray(b_qkv, np.float32)
    w_o = np.asarray(w_o, np.float32)
    b_o = np.asarray(b_o, np.float32)
    w_res = np.asarray(w_res, np.float32)
    b_res = np.asarray(b_res, np.float32)

    d = np.arange(32)
    qrows = np.concatenate([96 * h + d for h in range(H)])        # (256,)
    krows = np.concatenate([96 * h + 32 + d for h in range(H)])
    vrows = np.concatenate([96 * h + 64 + d for h in range(H)])

    Wq = w_qkv[qrows] * SCALE                                     # (256, C)
    Wk = w_qkv[krows]
    wqk = np.stack([Wq[:128].T, Wq[128:].T, Wk[:128].T, Wk[128:].T], axis=1)
    bqk = np.stack([b_qkv[qrows[:128]], b_qkv[qrows[128:]]], axis=1) * SCALE
    wv = np.ascontiguousarray(w_qkv[vrows].T)                     # (C, 256)
    wo = np.stack([w_o[:, :128].T, w_o[:, 128:].T, w_res.T], axis=1)
    bv = b_qkv[vrows]
    bout = (b_o + b_res + w_o @ bv)[:, None]

    import ml_dtypes
    bf = ml_dtypes.bfloat16
    return {
        "wqk": np.ascontiguousarray(wqk, bf),
        "bqk": np.ascontiguousarray(bqk, np.float32),
        "wv": np.ascontiguousarray(wv, bf),
        "wo": np.ascontiguousarray(wo, bf),
        "bout": np.ascontiguousarray(bout, np.float32),
    }


def make_in_maps(x, weights):
    import ml_dtypes
    x = np.ascontiguousarray(np.asarray(x).astype(ml_dtypes.bfloat16))
    return [
        dict(x_sh=np.ascontiguousarray(x[BLOC * i : BLOC * i + BLOC]), **weights)
        for i in range(NCORES)
    ]


class Runner:
    """Persistent PJRT executable for the SPMD bass program (axon path).

    Mirrors concourse.bass2jax.run_bass_via_pjrt's multi-core branch, but keeps
    the jitted callable so repeated executions don't re-trace/re-compile —
    needed both for a fast kernel() and for timing loops in test.py.
    """

    def __init__(self, nc=None, donate=True):
        import jax
        import concourse.mybir as mybir
        from concourse import bass2jax
        from jax.experimental.shard_map import shard_map
        from jax.sharding import Mesh, PartitionSpec

        if nc is None:
            nc = get_nc()
        bass2jax.install_neuronx_cc_hook()

        in_names, out_names, out_avals = [], [], []
        partition_name = (
            nc.partition_id_tensor.name if nc.partition_id_tensor else None
        )
        for alloc in nc.m.functions[0].allocations:
            if not isinstance(alloc, mybir.MemoryLocationSet):
                continue
            name = alloc.memorylocations[0].name
            if alloc.kind == "ExternalInput":
                if name != partition_name:
                    in_names.append(name)
            elif alloc.kind == "ExternalOutput":
                shape = tuple(alloc.tensor_shape)
                dtype = mybir.dt.np(alloc.dtype)
                out_avals.append(jax.core.ShapedArray(shape, dtype))
                out_names.append(name)
        n_params = len(in_names)
        n_outs = len(out_avals)
        all_in_names = list(in_names) + list(out_names)
        if partition_name is not None:
            all_in_names.append(partition_name)
        self.in_names = in_names
        self.out_names = out_names
        self.out_avals = out_avals

        donate_idx = tuple(range(n_params, n_params + n_outs)) if donate else ()

        def _body(*args):
            operands = list(args)
            if partition_name is not None:
                operands.append(bass2jax.partition_id_tensor())
            outs = bass2jax._bass_exec_p.bind(
                *operands,
                out_avals=tuple(out_avals),
                in_names=tuple(all_in_names),
                out_names=tuple(out_names),
                lowering_input_output_aliases=(),
                sim_require_finite=True,
                sim_require_nnan=True,
                nc=nc,
            )
            return tuple(outs)

        devices = jax.devices()[:NCORES]
        assert len(devices) == NCORES
        mesh = Mesh(np.asarray(devices), ("core",))
        in_specs = (PartitionSpec("core"),) * (n_params + n_outs)
        out_specs = (PartitionSpec("core"),) * n_outs
        self.sharded = jax.jit(
            shard_map(_body, mesh=mesh, in_specs=in_specs, out_specs=out_specs,
                      check_rep=False),
            donate_argnums=donate_idx,
            keep_unused=True,
        )
        self.mesh = mesh

    def prep(self, in_maps):
        return [
            np.concatenate([np.asarray(m[name]) for m in in_maps], axis=0)
            for name in self.in_names
        ]

    def zeros(self):
        return [
            np.zeros((NCORES * a.shape[0], *a.shape[1:]), a.dtype)
            for a in self.out_avals
        ]

    def call_async(self, concat_in):
        return self.sharded(*concat_in, *self.zeros())

    def __call__(self, in_maps):
        outs = self.call_async(self.prep(in_maps))
        arr = np.asarray(outs[0])
        return arr.reshape(NCORES, *self.out_avals[0].shape)


def get_runner():
    if "runner" not in _CACHE:
        _CACHE["runner"] = Runner()
    return _CACHE["runner"]


def run(x, weights, **kw):
    runner = get_runner()
    per_core = runner(make_in_maps(x, weights))
    out = per_core.reshape(B, C, L)
    return out, None


def kernel(x, w_qkv, b_qkv, w_o, b_o, w_res, b_res):
    weights = prep_weights(w_qkv, b_qkv, w_o, b_o, w_res, b_res)
    out, _ = run(x, weights)
    return out
